# revision 35
# baseline (speedup 1.0000x reference)
"""Trainium2 Bass kernel for nn_KrabbyPatty: batched NMF with MLP bread.

Per-core program (pure data parallel, one batch element per core):
  X  = relu(Xin @ W1 + b1)                  # [4096, 1024]
  D, C = D_init, C_init
  repeat 6x:
    C = C * (D^T X) / (D^T D C + eps)
    D = D * (X C^T) / (D C C^T + eps)
  out = D @ (C @ W2) + b2

Key layout/engine choices:
  - Host prep (layout only): Xin transposed + cast bf16; weights cast bf16
    and chunk-major; D/C initial states pre-arranged in device layouts.
  - XT = X^T [dout-part, l] computed by dense bf16 matmuls (PE), relu+bias
    fused on ScalarE during PSUM->SBUF eviction.
  - XB = X natural built from XT with DMA x-bar transposes (bf16,
    SBUF->SBUF) - zero PE/DVE cost.
  - D state in "dt4" layout [128=(4 l-quarters x 32r), 1024 l'] = folded
    D^T; C state in "c4c" layout [128=(4 d-quarters x 32r), 256 d'].
    All elementwise updates run on full 128 partitions.
  - D^T X and X C^T use 4x column-tiled matmuls (tile_position=(0,32b)):
    four concurrent rhs streams -> ~4x fewer PE cycles per X pass, and
    the quarter-partials land directly in the dt4/c4c layouts (the
    "reduction" is PSUM accumulation - no cross-partition sums needed).
  - D C C^T and DtD C use diagonal 32x32 tiles (tile_position=(32a,32a)).
  - Division via Ln/Exp on ScalarE (nc.scalar Reciprocal is banned for
    accuracy); numerator products on DVE with PSUM in-place reuse.
  - dnat (D natural, lhsT for D^T X) and CT (C^T chunks, lhsT for C X^T)
    rebuilt each step with one small DMA transpose each.
  - Final: C2 = C@W2 (8k accum), out tiles = D@C2 with K=32 row-tiled
    matmuls interleaved across the 4 row groups; b2 added on DVE during
    PSUM eviction, hidden behind the output DMA.
"""

import sys
import numpy as np

L, B, DM, R, K_STEPS = 4096, 8, 1024, 32, 6
EPS = 1e-9
NL = L // 128   # 32 l-tiles
ND = DM // 128  # 8 d-chunks
NQ = 4          # l-quarters (1024 each)


def build_nc():
    import concourse.bacc as bacc
    from concourse.bass import _add_dep_helper
    import concourse.mybir as mybir
    import concourse.tile as tile

    f32 = mybir.dt.float32
    bf16 = mybir.dt.bfloat16
    AF = mybir.ActivationFunctionType
    ALU = mybir.AluOpType

    class _Bacc(bacc.Bacc):
        """Pin all activations to the one act-func table that contains
        every function this kernel uses (relu/ln/exp/copy/identity).

        The default per-instruction set choice alternates between the
        `natural_log` and `exp_and_others` tables, inserting a ~1.3us
        ACT_TABLE_LOAD on every Ln<->Exp<->Copy transition (4+ per NMF
        step).  `natural_log_exp_and_others` holds all of them at once;
        hiding my funcs from the other sets (list order/length kept, so
        act_func_set_id indices still match act_info.json) makes the
        fixpoint hoist a single load for the whole kernel.
        """

        _KEEP = "natural_log_exp_and_others"

        def insert_act_table_loads(self):
            from concourse.hw_specs import get_activation_tables
            import bass_rust as _br

            has_activation = any(
                isinstance(i, mybir.InstActivation)
                for b in self.main_func.blocks
                for i in b.instructions
            )
            if not has_activation:
                return
            used = {
                i.func
                for b in self.main_func.blocks
                for i in b.instructions
                if isinstance(i, mybir.InstActivation)
            }
            tables = list(get_activation_tables(self.m.arch).items())
            keep_funcs = dict(tables)[self._KEEP]
            if used <= keep_funcs:
                tables = [
                    (name, funcs if name == self._KEEP else funcs - used)
                    for name, funcs in tables
                ]
            _br.insert_act_table_loads(self, tables)

    nc = _Bacc()
    xt_in = nc.dram_tensor("xt_in", [DM, L], bf16, kind="ExternalInput")
    w1s_in = nc.dram_tensor("w1s", [128, ND, DM], bf16, kind="ExternalInput")
    w2s_in = nc.dram_tensor("w2s", [128, ND, DM], bf16, kind="ExternalInput")
    b1s_in = nc.dram_tensor("b1s", [128, ND], f32, kind="ExternalInput")
    b2f_in = nc.dram_tensor("b2f", [128, DM], f32, kind="ExternalInput")
    dt4_in = nc.dram_tensor("dt4", [128, L // 4], f32, kind="ExternalInput")
    dt4b_in = nc.dram_tensor("dt4b", [128, L // 4], bf16, kind="ExternalInput")
    dnat_in = nc.dram_tensor("dnat0", [128, ND, 128], bf16, kind="ExternalInput")
    c4c_in = nc.dram_tensor("c4c", [128, DM // 4], f32, kind="ExternalInput")
    c4cb_in = nc.dram_tensor("c4cb", [128, DM // 4], bf16, kind="ExternalInput")
    out = nc.dram_tensor("out", [L, DM], f32, kind="ExternalOutput")

    with tile.TileContext(nc) as tc:
        with (
            tc.tile_pool(name="bigsb", bufs=1) as bigsb,    # xt / xb
            tc.tile_pool(name="wpool", bufs=1) as wpool,    # w1s then w2s
            tc.tile_pool(name="xq", bufs=2) as xqp,         # Xin^T quarter staging
            tc.tile_pool(name="state", bufs=1) as st,       # D/C state + consts
            tc.tile_pool(name="work", bufs=1) as wk,        # per-step recompute
            tc.tile_pool(name="big", bufs=2, space="PSUM") as psbig,    # [128,1024]
            tc.tile_pool(name="small", bufs=3, space="PSUM") as pssm,   # [128,256]
            tc.tile_pool(name="flr", bufs=1, space="PSUM") as flrp,     # HAM filler
            tc.tile_pool(name="ot3", bufs=1) as ot3p,       # 3rd out-staging slot
        ):
            # ---------------- constants / initial state ----------------
            b1s = st.tile([128, ND], f32, tag="b1s")
            nc.sync.dma_start(b1s[:], b1s_in[:, :])
            eps_c = st.tile([128, 1], f32, tag="epsc")
            nc.vector.memset(eps_c[:], EPS)

            # NMF-start-critical state on the gpsimd (SWDGE) ring; small,
            # so the phase-1 ramp DMAs keep nearly all SDMA bandwidth
            c4c_f = st.tile([128, DM // 4], f32, tag="c4cf")
            _e1 = nc.gpsimd.dma_start(c4c_f[:], c4c_in[:, :])
            c4c_b = st.tile([128, DM // 4], bf16, tag="c4cb")
            _e2 = nc.gpsimd.dma_start(c4c_b[:], c4cb_in[:, :])
            dnat = st.tile([128, ND, 128], bf16, tag="dnat")
            _e3 = nc.gpsimd.dma_start(dnat[:], dnat_in[:, :, :])

            # ---------------- phase 1: XT = relu(W1^T Xin^T + b1) ------
            # xt[p, j, l] = X[l, 128j + p];  xb[p, i, d] = X[128i + p, d]
            xt = bigsb.tile([128, ND, L], bf16, tag="xt")
            xb = bigsb.tile([128, NL, DM], bf16, tag="xb")
            w1s = wpool.tile([128, ND, DM], bf16, tag="wts")

            # HAM warmth filler: tiny throwaway matmuls issued into PE-idle
            # dependency windows. The PE clock gate (HAM) drops to 1.2 GHz
            # after ~3.4us of idle; the NMF's C/D update windows exceed
            # that, so without filler every step's matmuls run at half
            # clock (measured: 400-600ns per MM instead of ~110-215ns).
            _flr_cnt = [0]

            def pe_filler(n):
                p = flrp.tile([32, 128], f32, tag="flr",
                              name=f"flr{_flr_cnt[0]}")
                _flr_cnt[0] += 1
                for _ in range(n):
                    nc.tensor.matmul(p[:], xt[:, 0, 0:32], xt[:, 0, 0:128],
                                     start=True, stop=True)

            nc.sync.dma_start(w1s[:], w1s_in[:, :, :])
            for q in range(NQ):           # l-quarter
                qi = q
                xq = xqp.tile([128, ND, 1024], bf16, tag="xq")
                for k in range(ND):
                    nc.sync.dma_start(
                        xq[:, k, :],
                        xt_in[128 * k:128 * (k + 1), 1024 * q:1024 * (q + 1)])
                for lb in range(2):       # 512-block within quarter
                    for j in range(ND):   # dout tile
                        pm = psbig.tile([128, 512], f32, tag="pbig",
                                        name=f"p1_{q}_{lb}_{j}")
                        for k in range(ND):
                            nc.tensor.matmul(
                                pm[:],
                                w1s[:, k, 128 * j:128 * (j + 1)],
                                xq[:, k, 512 * lb:512 * (lb + 1)],
                                start=(k == 0), stop=(k == ND - 1))
                        lo = 1024 * q + 512 * lb
                        _relu = nc.scalar.activation(
                            xt[:, j, lo:lo + 512], pm[:],
                            AF.Relu, bias=b1s[:, j:j + 1], scale=1.0)
                        if q == 1 and lb == 1 and j == ND - 1:
                            gate_q1 = _relu
                        # xb transpose for column block j; in the last
                        # quarter fire per-j on alternating rings so
                        # the tail shrinks to one call's latency
                        if lb == 1:
                            eng = nc.scalar if (qi == 3 and j % 2 == 1) \
                                else nc.sync
                            eng.dma_start_transpose(
                                xb[:, 8 * q:8 * q + 8,
                                   128 * j:128 * (j + 1)],
                                xt[:, j, 1024 * q:1024 * (q + 1)])

            # ---------------- phase 2: NMF steps ------------------------
            # remaining state (first consumed ~20us into the NMF): emitted
            # after phase-1 so these SWDGE copies run mid-phase-1 at the
            # earliest scheduler tick with zero ramp competition
            b2f = st.tile([128, DM], f32, tag="b2f")
            _i1 = nc.gpsimd.dma_start(b2f[:], b2f_in[:, :])
            dt4_f = st.tile([128, L // 4], f32, tag="dt4f")
            _i2 = nc.gpsimd.dma_start(dt4_f[:], dt4_in[:, :])
            dt4_b = st.tile([128, L // 4], bf16, tag="dt4b")
            _i3 = nc.gpsimd.dma_start(dt4_b[:], dt4b_in[:, :])
            for _i in (_i1, _i2, _i3, _e1, _e2, _e3):
                _add_dep_helper(_i.ins, gate_q1.ins, sync=True,
                                reason="hold state loads out of the DMA ramp")

            i_order0 = list(range(NL))
            i_orderN = ([i for i in range(NL) if i % 8 < 4]
                        + [i for i in range(NL) if i % 8 >= 4])
            for s in range(K_STEPS):
                # --- DtD [32,32] = sum_i dnat_i^T dnat_i
                i_order = i_order0 if s == 0 else i_orderN
                p_dtd = pssm.tile([32, 32], f32, tag="psm", name=f"dtd{s}")
                for n, i in enumerate(i_order):
                    lhsT = dnat[:, i % 8, 32 * (i // 8):32 * (i // 8) + 32]
                    nc.tensor.matmul(p_dtd[:], lhsT, lhsT,
                                     start=(n == 0), stop=(n == NL - 1))

                # --- DtD replicated to 4 partition quarters (bf16 lhsT)
                dtd4 = wk.tile([128, 32], bf16, tag="dtd4", name=f"dtd4_{s}")
                for b in range(4):
                    nc.any.tensor_copy(dtd4[32 * b:32 * b + 32, :], p_dtd[:])

                # --- DtDC in c4c layout: diagonal 32x32 tiles
                p_dc = pssm.tile([128, 256], f32, tag="psm", name=f"dc{s}")
                for b in range(4):
                    nc.tensor.matmul(
                        p_dc[32 * b:32 * b + 32, :],
                        dtd4[32 * b:32 * b + 32, :],
                        c4c_b[32 * b:32 * b + 32, :],
                        start=True, stop=True,
                        tile_position=(32 * b, 32 * b))

                if s == 0:
                    pe_filler(110)  # cover the tail-transpose wait warm

                # --- DtX in c4c layout: [(b,r), d'] col-tiled 4x
                p_dtx = pssm.tile([128, 256], f32, tag="psm", name=f"dtx{s}")
                for n, i in enumerate(i_order):
                    lhsT = dnat[:, i % 8, 32 * (i // 8):32 * (i // 8) + 32]
                    for b in range(4):
                        nc.tensor.matmul(
                            p_dtx[32 * b:32 * b + 32, :], lhsT,
                            xb[:, i, 256 * b:256 * (b + 1)],
                            start=(n == 0), stop=(n == NL - 1),
                            tile_position=(0, 32 * b))

                pe_filler(80)       # keep PE warm through the C window

                # --- C update: C *= DtX / (DtDC + eps), in d'-halves so
                # each half's CT transpose fires as soon as it's ready
                recip_c = wk.tile([128, 256], f32, tag="rc", name=f"rc{s}")
                ct2 = wk.tile([128, 2, 128], bf16, tag="ct", name=f"ct{s}")
                for m in range(2):
                    sl = slice(128 * m, 128 * (m + 1))
                    nc.scalar.activation(p_dc[:, sl], p_dc[:, sl],
                                         AF.Ln, bias=eps_c[:, 0:1])
                    nc.scalar.activation(recip_c[:, sl], p_dc[:, sl],
                                         AF.Exp, scale=-1.0)
                    nc.vector.tensor_tensor(
                        out=p_dtx[:, sl], in0=c4c_f[:, sl],
                        in1=p_dtx[:, sl], op=ALU.mult)
                    nc.vector.tensor_tensor(
                        out=c4c_f[:, sl], in0=p_dtx[:, sl],
                        in1=recip_c[:, sl], op=ALU.mult)
                    nc.vector.tensor_copy(c4c_b[:, sl], c4c_f[:, sl])
                    # chunk k of C^T lives at ct2[:, k%2, 32*(k//2):+32];
                    # half m yields the k%2==m chunks
                    eng = nc.sync if m == 0 else nc.scalar
                    eng.dma_start_transpose(
                        ct2[:, m:m + 1, :], c4c_b[:, sl])

                # --- CCt [32,32] = sum_k CT_k^T CT_k (even chunks first)
                k_order = [0, 2, 4, 6, 1, 3, 5, 7]
                p_cct = pssm.tile([32, 32], f32, tag="psm", name=f"cct{s}")
                for n, k in enumerate(k_order):
                    lhsT = ct2[:, k % 2, 32 * (k // 2):32 * (k // 2) + 32]
                    nc.tensor.matmul(p_cct[:], lhsT, lhsT,
                                     start=(n == 0), stop=(n == ND - 1))
                cct4 = wk.tile([128, 32], bf16, tag="cct4", name=f"cct4_{s}")
                for b in range(4):
                    nc.any.tensor_copy(cct4[32 * b:32 * b + 32, :], p_cct[:])

                # --- XCt in dt4 layout: [(a,r), l'] col-tiled 4x over l-quarters
                p_xct = psbig.tile([128, 1024], f32, tag="pbig", name=f"xct{s}")
                for n, k in enumerate(k_order):
                    lhsT = ct2[:, k % 2, 32 * (k // 2):32 * (k // 2) + 32]
                    for h in range(2):
                        for a in range(4):
                            lo = 1024 * a + 512 * h
                            nc.tensor.matmul(
                                p_xct[32 * a:32 * a + 32, 512 * h:512 * (h + 1)],
                                lhsT, xt[:, k, lo:lo + 512],
                                start=(n == 0), stop=(n == ND - 1),
                                tile_position=(0, 32 * a))

                if s == K_STEPS - 1:
                    # w2 load (reuses w1 slot; no DMA transposes remain, so
                    # no xbar-mode serialization risk) + C2 = C@W2 + b2 prep:
                    # runs on PE/ACT while the final D update proceeds
                    w2s = wpool.tile([128, ND, DM], bf16, tag="wts")
                    _w2i = nc.sync.dma_start(w2s[:], w2s_in[:, :, :])
                    _add_dep_helper(_w2i.ins, gate_s4.ins, sync=True,
                                    reason="keep w2s off the NMF sync ring")
                    p_c2 = psbig.tile([32, 1024], f32, tag="pbig", name="c2")
                    for k in range(ND):
                        lhsT = ct2[:, k % 2, 32 * (k // 2):32 * (k // 2) + 32]
                        for h in range(2):
                            nc.tensor.matmul(
                                p_c2[:, 512 * h:512 * (h + 1)], lhsT,
                                w2s[:, k, 512 * h:512 * (h + 1)],
                                start=(k == 0), stop=(k == ND - 1))
                    c2b4 = st.tile([128, DM], bf16, tag="c2b4")
                    for b in range(4):
                        nc.any.tensor_copy(
                            c2b4[32 * b:32 * b + 32, :], p_c2[:])

                # --- DCCt in dt4 layout: diagonal tiles
                p_dcc = psbig.tile([128, 1024], f32, tag="pbig", name=f"dcc{s}")
                for h in range(2):
                    for a in range(4):
                        nc.tensor.matmul(
                            p_dcc[32 * a:32 * a + 32, 512 * h:512 * (h + 1)],
                            cct4[32 * a:32 * a + 32, :],
                            dt4_b[32 * a:32 * a + 32, 512 * h:512 * (h + 1)],
                            start=True, stop=True,
                            tile_position=(32 * a, 32 * a))

                pe_filler(100)      # keep PE warm through the D window

                # --- D update: D *= XCt / (DCCt + eps), split in halves so
                # ACT (Ln/Exp), DVE (muls/cast) and DMA (dnat transpose)
                # pipeline against each other.
                recip_d = wk.tile([128, 1024], f32, tag="rd", name=f"rd{s}")
                if s < K_STEPS - 1:
                    dnat = st.tile([128, ND, 128], bf16, tag="dnat")
                for h in range(2):
                    sl = slice(512 * h, 512 * (h + 1))
                    nc.scalar.activation(p_dcc[:, sl], p_dcc[:, sl],
                                         AF.Ln, bias=eps_c[:, 0:1])
                    nc.scalar.activation(recip_d[:, sl], p_dcc[:, sl],
                                         AF.Exp, scale=-1.0)
                    nc.vector.tensor_tensor(
                        out=p_xct[:, sl], in0=dt4_f[:, sl],
                        in1=p_xct[:, sl], op=ALU.mult)
                    nc.vector.tensor_tensor(
                        out=dt4_f[:, sl], in0=p_xct[:, sl],
                        in1=recip_d[:, sl], op=ALU.mult)
                    nc.vector.tensor_copy(dt4_b[:, sl], dt4_f[:, sl])
                    # rebuild D natural (lhsT for next step's DtX/DtD)
                    if s < K_STEPS - 1:
                        _dn = nc.sync.dma_start_transpose(
                            dnat[:, 4 * h:4 * h + 4, :], dt4_b[:, sl])
                        if s == K_STEPS - 2 and h == 1:
                            gate_s4 = _dn

            # ---------------- phase 3: out = D @ C2 ----------
            # out tiles: l-tile i = 8a + step handled by K=32 row group a;
            # 4 tiles (one per row group, concurrent on PE) are batched
            # into one 2 MB output DMA. ot slots reuse the xq pool.
            out_v = out.rearrange("(a s p) d -> p a s d", a=4, p=128)
            for step in range(8):
                for g in range(2):
                    pool = ot3p if (2 * step + g) % 3 == 2 else xqp
                    ot = pool.tile([128, 2, DM], f32, tag="xq",
                                   name=f"ot{step}_{g}")
                    for u in range(2):
                        a = 2 * g + u
                        po = psbig.tile([128, 1024], f32, tag="pbig",
                                        name=f"po{step}_{a}")
                        for h in range(2):
                            nc.tensor.matmul(
                                po[:, 512 * h:512 * (h + 1)],
                                dt4_b[32 * a:32 * a + 32,
                                      128 * step:128 * (step + 1)],
                                c2b4[32 * a:32 * a + 32,
                                     512 * h:512 * (h + 1)],
                                start=True, stop=True,
                                tile_position=(32 * a, 0))
                        nc.vector.tensor_tensor(
                            out=ot[:, u, :], in0=po[:], in1=b2f[:],
                            op=ALU.add)
                    nc.sync.dma_start(
                        out_v[:, 2 * g:2 * g + 2, step, :], ot[:, :, :])

    nc.finalize()
    return nc


def prep_in_maps(inputs):
    """Host-side layout prep (transpose/cast/tile only - no FLOPs)."""
    import ml_dtypes
    bf16 = ml_dtypes.bfloat16

    x = np.asarray(inputs["input_tensor"], np.float32)       # [L, B, DM]
    w1 = np.asarray(inputs["W1"], np.float32)
    w2 = np.asarray(inputs["W2"], np.float32)
    b1 = np.asarray(inputs["b1"], np.float32)
    b2 = np.asarray(inputs["b2"], np.float32)
    d0 = np.asarray(inputs["D_init"], np.float32)            # [L, R]
    c0 = np.asarray(inputs["C_init"], np.float32)            # [R, DM]

    def chunk_major(w):  # [DM, DM] -> [128, ND, DM]
        return np.ascontiguousarray(
            w.reshape(ND, 128, DM).transpose(1, 0, 2).astype(bf16))

    # dt4[(a,r), l'] = D^T[r, 1024a + l']
    dt4 = np.ascontiguousarray(
        d0.reshape(4, 1024, R).transpose(0, 2, 1).reshape(128, 1024))
    # dnat0[p, m, 32a + r] = D[128(8a + m) + p, r]
    dnat0 = np.ascontiguousarray(
        d0.reshape(4, 8, 128, R).transpose(2, 1, 0, 3).reshape(128, 8, 128)
        .astype(bf16))
    # c4c[(b,r), d'] = C[r, 256b + d']
    c4c = np.ascontiguousarray(
        c0.reshape(R, 4, 256).transpose(1, 0, 2).reshape(128, 256))

    shared = {
        "w1s": chunk_major(w1),
        "w2s": chunk_major(w2),
        "b1s": np.ascontiguousarray(b1.reshape(ND, 128).T),
        "b2f": np.ascontiguousarray(np.tile(b2.reshape(1, DM), (128, 1))),
        "dt4": dt4,
        "dt4b": np.ascontiguousarray(dt4.astype(bf16)),
        "dnat0": dnat0,
        "c4c": c4c,
        "c4cb": np.ascontiguousarray(c4c.astype(bf16)),
    }
    in_maps = []
    for b in range(B):
        xt_b = np.ascontiguousarray(x[:, b, :].T.astype(bf16))  # [DM, L]
        in_maps.append({"xt_in": xt_b, **shared})
    return in_maps


_NC_CACHE = None


def _kernel_numpy(inputs):
    """Correct host fallback (only if the Bass path fails)."""
    X0 = np.transpose(np.asarray(inputs["input_tensor"], np.float32), (1, 0, 2))
    W1 = np.asarray(inputs["W1"], np.float32); b1 = np.asarray(inputs["b1"], np.float32)
    W2 = np.asarray(inputs["W2"], np.float32); b2 = np.asarray(inputs["b2"], np.float32)
    outs = []
    for b in range(B):
        X = np.maximum(X0[b] @ W1 + b1, 0.0)
        D = np.asarray(inputs["D_init"], np.float32).copy()
        C = np.asarray(inputs["C_init"], np.float32).copy()
        for _ in range(K_STEPS):
            C = C * (D.T @ X) / ((D.T @ D) @ C + EPS)
            D = D * (X @ C.T) / (D @ (C @ C.T) + EPS)
        outs.append((D @ C) @ W2 + b2)
    return np.stack(outs, axis=0).transpose(1, 0, 2).astype(np.float32)


def kernel(**inputs) -> np.ndarray:
    global _NC_CACHE
    try:
        from concourse.bass_utils import run_bass_kernel_spmd

        if _NC_CACHE is None:
            _NC_CACHE = build_nc()
        in_maps = prep_in_maps(inputs)
        res = run_bass_kernel_spmd(_NC_CACHE, in_maps, core_ids=list(range(B)))
        outs = [res.results[b]["out"] for b in range(B)]
        return np.stack(outs, axis=1).astype(np.float32)  # [L, B, DM]
    except Exception as e:
        print(f"kernel: Bass path failed ({type(e).__name__}: {e}); "
              f"falling back to numpy", file=sys.stderr)
        return _kernel_numpy(inputs)
